# revision 2
# baseline (speedup 1.0000x reference)
"""MoE experts kernel (GPT-OSS style, dense routing over 8 experts) for 8 trn2 NeuronCores.

Strategy: expert-parallel. Core e computes its expert's full MLP for all 4096
tokens, scaled by that expert's routing weight column; the host sums the 8
partial outputs (the unshard step for expert-parallel sharding).

Everything runs in a transposed [feature, token] layout on-chip so that no
transposes are needed anywhere:
  gate   = Wg^T @ X^T          (Wg [H,D] natural = lhsT, X^T [H,T] natural = rhs)
  up     = Wu^T @ X^T
  act    = (up + bu + 1) * gelu_sigmoid(gate + bg)     [D, T] fp16
  out    = (act^T @ Wd + ones^T bd) * w_route          (act tile = lhsT, Wd = rhs)
giving out in [T, H] layout directly.

Matmuls run in fp16 (4x the mantissa of bf16, same PE speed; all values here
are O(10) so fp16 range is ample). PSUM accumulation is fp32.
"""

import os
import numpy as np

import concourse.bass as bass
import concourse.mybir as mybir
import concourse.tile as tile
from concourse import bacc
from concourse.bass import ts, ds
from concourse.bass_utils import run_bass_kernel_spmd

AF = mybir.ActivationFunctionType
OP = mybir.AluOpType
F16 = mybir.dt.float16
F32 = mybir.dt.float32

P = 128
H = 1024          # hidden dim
D = 1024          # expert dim
NUM_EXPERTS = 8

# Gelu_apprx_sigmoid LUT computes x*sigmoid(1.702x) in one ACT pass.
# Fallback (False) builds it from Sigmoid + 2 DVE ops (simulator-friendly).
USE_GELU_LUT = True


def build_nc(T=4096, use_gelu_lut=USE_GELU_LUT):
    KT = H // P            # k-tiles for gate/up matmul (contraction over H)
    KD = D // P            # k-tiles for down matmul (contraction over D)
    DT = D // P            # d-tiles of the expert dim
    TCH = 512              # token chunk = psum free dim
    NCH = T // TCH         # token chunks
    TTILES = TCH // P      # 128-token tiles per chunk
    HCH = 512              # h chunk of the down matmul output
    NHCH = H // HCH

    nc = bacc.Bacc("TRN2", debug=False, enable_asserts=False, num_devices=NUM_EXPERTS)

    xt_d = nc.dram_tensor("xt", [H, T], F16, kind="ExternalInput")
    wg_d = nc.dram_tensor("wg", [H, D], F16, kind="ExternalInput")
    wu_d = nc.dram_tensor("wu", [H, D], F16, kind="ExternalInput")
    wd_d = nc.dram_tensor("wd", [D, H], F16, kind="ExternalInput")
    bg_d = nc.dram_tensor("bg", [P, DT], F32, kind="ExternalInput")
    bu1_d = nc.dram_tensor("bu1", [P, DT], F32, kind="ExternalInput")
    bd_d = nc.dram_tensor("bd", [1, H], F16, kind="ExternalInput")
    wr_d = nc.dram_tensor("wr", [P, T // P], F32, kind="ExternalInput")
    out_d = nc.dram_tensor("out", [T, H], F32, kind="ExternalOutput")
    out_ap = out_d.ap()

    with tile.TileContext(nc) as tc:
        with (
            tc.tile_pool(name="wpool", bufs=1) as wpool,
            tc.tile_pool(name="xpool", bufs=3) as xpool,
            tc.tile_pool(name="gpool", bufs=3) as gpool,
            tc.tile_pool(name="apool", bufs=2) as apool,
            tc.tile_pool(name="opool", bufs=4) as opool,
            tc.tile_pool(name="pgu", bufs=2, space="PSUM") as pgu,
            tc.tile_pool(name="pdn", bufs=3, space="PSUM") as pdn,
        ):
            # --- resident weights/biases ---
            wg_sb = wpool.tile([P, KT, D], F16, name="wg_sb")
            nc.sync.dma_start(wg_sb[:], wg_d.ap().rearrange("(ko p) d -> p ko d", p=P))
            wu_sb = wpool.tile([P, KT, D], F16, name="wu_sb")
            nc.sync.dma_start(wu_sb[:], wu_d.ap().rearrange("(ko p) d -> p ko d", p=P))
            wd_sb = wpool.tile([P, KD, H], F16, name="wd_sb")
            nc.sync.dma_start(wd_sb[:], wd_d.ap().rearrange("(ko p) h -> p ko h", p=P))
            bg_sb = wpool.tile([P, DT], F32, name="bg_sb")
            nc.sync.dma_start(bg_sb[:], bg_d.ap())
            bu1_sb = wpool.tile([P, DT], F32, name="bu1_sb")
            nc.sync.dma_start(bu1_sb[:], bu1_d.ap())
            bd_sb = wpool.tile([1, H], F16, name="bd_sb")
            nc.sync.dma_start(bd_sb[:], bd_d.ap())
            wr_sb = wpool.tile([P, T // P], F32, name="wr_sb")
            nc.sync.dma_start(wr_sb[:], wr_d.ap())
            ones_sb = wpool.tile([1, P], F16, name="ones_sb")
            nc.vector.memset(ones_sb[:], 1.0)

            act_tiles = [None] * NCH

            def emit_gateup(c, xt_sb):
                act_t = apool.tile([P, DT, TCH], F16, name="act_t")
                act_tiles[c] = act_t
                for dd in range(DT):
                    pg = pgu.tile([P, TCH], F32, name="pg", bufs=2)
                    for k in range(KT):
                        nc.tensor.matmul(
                            pg[:], wg_sb[:, k, ts(dd, P)], xt_sb[:, k, :],
                            start=(k == 0), stop=(k == KT - 1),
                        )
                    pu = pgu.tile([P, TCH], F32, name="pu", bufs=2)
                    for k in range(KT):
                        nc.tensor.matmul(
                            pu[:], wu_sb[:, k, ts(dd, P)], xt_sb[:, k, :],
                            start=(k == 0), stop=(k == KT - 1),
                        )
                    glu_t = gpool.tile([P, TCH], F16, name="glu_t")
                    if use_gelu_lut:
                        # glu = g*sigmoid(1.702 g), g = psum_gate + bg
                        nc.scalar.activation(
                            glu_t[:], pg[:], AF.Gelu_apprx_sigmoid,
                            bias=bg_sb[:, dd:dd + 1], scale=1.0,
                        )
                    else:
                        g_t = gpool.tile([P, TCH], F32, name="g_t")
                        nc.vector.tensor_scalar(
                            g_t[:], pg[:], bg_sb[:, dd:dd + 1], None, OP.add,
                        )
                        s_t = gpool.tile([P, TCH], F16, name="s_t")
                        nc.scalar.activation(s_t[:], g_t[:], AF.Sigmoid, scale=1.702)
                        nc.vector.tensor_mul(glu_t[:], g_t[:], s_t[:])
                    # act = (psum_up + (bu+1)) * glu
                    nc.vector.scalar_tensor_tensor(
                        act_t[:, dd, :], pu[:], bu1_sb[:, dd:dd + 1], glu_t[:],
                        OP.add, OP.mult,
                    )

            def emit_down(c):
                act_t = act_tiles[c]
                for tt in range(TTILES):
                    tcol = c * TTILES + tt
                    for hh in range(NHCH):
                        po = pdn.tile([P, HCH], F32, name="po", bufs=3)
                        # bias row: out[t, h] += 1 * bd[h]  (K=1 matmul)
                        nc.tensor.matmul(
                            po[:], ones_sb[:], bd_sb[:, ts(hh, HCH)],
                            start=True, stop=False,
                        )
                        for kd in range(KD):
                            nc.tensor.matmul(
                                po[:], act_t[:, kd, ts(tt, P)], wd_sb[:, kd, ts(hh, HCH)],
                                start=False, stop=(kd == KD - 1),
                            )
                        ot = opool.tile([P, HCH], F32, name="ot")
                        nc.vector.tensor_scalar(
                            ot[:], po[:], wr_sb[:, tcol:tcol + 1], None, OP.mult,
                        )
                        nc.sync.dma_start(
                            out_ap[ds(c * TCH + tt * P, P), ts(hh, HCH)], ot[:],
                        )

            xt3 = xt_d.ap().rearrange("(ko p) t -> p ko t", p=P)
            for c in range(NCH):
                xt_sb = xpool.tile([P, KT, TCH], F16, name="xt_sb")
                nc.sync.dma_start(xt_sb[:], xt3[:, :, ts(c, TCH)])
                emit_gateup(c, xt_sb)
                if c > 0:
                    emit_down(c - 1)
            emit_down(NCH - 1)

    nc.finalize()
    return nc


def make_in_maps(hidden_states, routing_weights, gate_up_proj, gate_up_proj_bias,
                 down_proj, down_proj_bias):
    T = hidden_states.shape[0]
    xt = np.ascontiguousarray(np.asarray(hidden_states, dtype=np.float32).T).astype(np.float16)
    gu = np.asarray(gate_up_proj, dtype=np.float32)
    gub = np.asarray(gate_up_proj_bias, dtype=np.float32)
    wd = np.asarray(down_proj, dtype=np.float32)
    bd = np.asarray(down_proj_bias, dtype=np.float32)
    wr = np.asarray(routing_weights, dtype=np.float32)

    in_maps = []
    for e in range(NUM_EXPERTS):
        in_maps.append({
            "xt": xt,
            "wg": np.ascontiguousarray(gu[e, :, 0::2]).astype(np.float16),
            "wu": np.ascontiguousarray(gu[e, :, 1::2]).astype(np.float16),
            "wd": np.ascontiguousarray(wd[e]).astype(np.float16),
            "bg": np.ascontiguousarray(gub[e, 0::2].reshape(D // P, P).T),
            "bu1": np.ascontiguousarray((gub[e, 1::2] + 1.0).reshape(D // P, P).T),
            "bd": bd[e].reshape(1, H).astype(np.float16),
            "wr": np.ascontiguousarray(wr[:, e].reshape(T // P, P).T),
        })
    return in_maps


_NC_CACHE = {}


def _get_nc(T=4096):
    if T not in _NC_CACHE:
        _NC_CACHE[T] = build_nc(T)
    return _NC_CACHE[T]


def run(inputs, trace=False, trace_cores=None, **kwargs):
    """Build (cached), run on 8 cores, return (full_output, BassKernelResults)."""
    T = inputs["hidden_states"].shape[0]
    nc = _get_nc(T)
    in_maps = make_in_maps(**inputs)
    res = run_bass_kernel_spmd(
        nc, in_maps, core_ids=list(range(NUM_EXPERTS)),
        trace=trace, trace_cores=trace_cores, **kwargs,
    )
    out = np.zeros((T, H), np.float32)
    for c in range(NUM_EXPERTS):
        out += res.results[c]["out"]
    return out, res


def kernel(hidden_states, routing_weights, gate_up_proj, gate_up_proj_bias,
           down_proj, down_proj_bias):
    out, _ = run(dict(
        hidden_states=np.asarray(hidden_states),
        routing_weights=np.asarray(routing_weights),
        gate_up_proj=np.asarray(gate_up_proj),
        gate_up_proj_bias=np.asarray(gate_up_proj_bias),
        down_proj=np.asarray(down_proj),
        down_proj_bias=np.asarray(down_proj_bias),
    ))
    return out


# revision 7
# speedup vs baseline: 1.0818x; 1.0818x over previous
"""MoE experts kernel (GPT-OSS style, dense routing over 8 experts) for 8 trn2 NeuronCores.

Strategy: expert-parallel. Core e computes its expert's full MLP for all 4096
tokens, scaled by that expert's routing weight column; the host sums the 8
partial outputs (the unshard step for expert-parallel sharding).

Everything runs in a transposed [feature, token] layout on-chip so that no
transposes are needed anywhere:
  gate   = Wg^T @ X^T          (Wg [H,D] natural = lhsT, X^T [H,T] natural = rhs)
  up     = Wu^T @ X^T
  act    = (up + bu + 1) * gelu_sigmoid(gate + bg)     [D, T] fp16
  out    = (act^T @ Wd + ones^T bd) * w_route          (act tile = lhsT, Wd = rhs)
giving out in [T, H] layout directly.

Matmuls run in fp16 (4x the mantissa of bf16, same PE speed; all values here
are O(10) so fp16 range is ample). PSUM accumulation is fp32.
"""

import os
import numpy as np

import concourse.bass as bass
import concourse.mybir as mybir
import concourse.tile as tile
from concourse import bacc
from concourse.bass import ts, ds
from concourse.bass_utils import run_bass_kernel_spmd

AF = mybir.ActivationFunctionType
OP = mybir.AluOpType
F16 = mybir.dt.float16
F32 = mybir.dt.float32

P = 128
H = 1024          # hidden dim
D = 1024          # expert dim
NUM_EXPERTS = 8

# Gelu_apprx_sigmoid LUT computes x*sigmoid(1.702x) in one ACT pass.
# Fallback (False) builds it from Sigmoid + 2 DVE ops (simulator-friendly).
USE_GELU_LUT = True


def build_nc(T=4096, use_gelu_lut=USE_GELU_LUT):
    KT = H // P            # k-tiles for gate/up matmul (contraction over H)
    KD = D // P            # k-tiles for down matmul (contraction over D)
    DT = D // P            # d-tiles of the expert dim
    TCH = 512              # token chunk = psum free dim
    NCH = T // TCH         # token chunks
    TTILES = TCH // P      # 128-token tiles per chunk
    HCH = 512              # h chunk of the down matmul output
    NHCH = H // HCH

    nc = bacc.Bacc("TRN2", debug=False, enable_asserts=False, num_devices=NUM_EXPERTS)

    xt_d = nc.dram_tensor("xt", [H, T], F16, kind="ExternalInput")
    wg_d = nc.dram_tensor("wg", [H, D], F16, kind="ExternalInput")
    wu_d = nc.dram_tensor("wu", [H, D], F16, kind="ExternalInput")
    wd_d = nc.dram_tensor("wd", [D, H], F16, kind="ExternalInput")
    bg_d = nc.dram_tensor("bg", [P, DT], F32, kind="ExternalInput")
    bu1_d = nc.dram_tensor("bu1", [P, DT], F32, kind="ExternalInput")
    bdb_d = nc.dram_tensor("bdb", [P, H], F32, kind="ExternalInput")
    wr_d = nc.dram_tensor("wr", [P, T // P], F32, kind="ExternalInput")
    out_d = nc.dram_tensor("out", [T, H], F32, kind="ExternalOutput")
    out_ap = out_d.ap()

    with tile.TileContext(nc) as tc:
        with (
            tc.tile_pool(name="wpool", bufs=1) as wpool,
            tc.tile_pool(name="xpool", bufs=3) as xpool,
            tc.tile_pool(name="gpool", bufs=3) as gpool,
            tc.tile_pool(name="apool", bufs=2) as apool,
            tc.tile_pool(name="opool", bufs=4) as opool,
            tc.tile_pool(name="pgu", bufs=2, space="PSUM") as pgu,
            tc.tile_pool(name="pdn", bufs=3, space="PSUM") as pdn,
        ):
            # --- resident weights/biases ---
            # DMA issue order is chosen so the dependencies of the FIRST
            # matmuls land first: small biases, xt chunk 0 (split per
            # k-tile), gate weights (split per k-tile), then up weights;
            # the down weights are deferred until after chunk 0's gate-up
            # is emitted (they are first consumed one full chunk later).
            bg_sb = wpool.tile([P, DT], F32, name="bg_sb")
            nc.sync.dma_start(bg_sb[:], bg_d.ap())
            bu1_sb = wpool.tile([P, DT], F32, name="bu1_sb")
            nc.sync.dma_start(bu1_sb[:], bu1_d.ap())
            wr_sb = wpool.tile([P, T // P], F32, name="wr_sb")
            nc.sync.dma_start(wr_sb[:], wr_d.ap())
            bdb_sb = wpool.tile([P, H], F32, name="bdb_sb")
            nc.sync.dma_start(bdb_sb[:], bdb_d.ap())

            wg_sb = wpool.tile([P, KT, D], F16, name="wg_sb")
            wu_sb = wpool.tile([P, KT, D], F16, name="wu_sb")
            wd_sb = wpool.tile([P, KD, H], F16, name="wd_sb")
            wg3 = wg_d.ap().rearrange("(ko p) d -> p ko d", p=P)
            wu3 = wu_d.ap().rearrange("(ko p) d -> p ko d", p=P)
            wd3 = wd_d.ap().rearrange("(ko p) h -> p ko h", p=P)

            act_tiles = [None] * NCH

            def emit_gateup(c, xt_sb):
                act_t = apool.tile([P, DT, TCH], F16, name="act_t")
                act_tiles[c] = act_t
                for dd in range(DT):
                    pg = pgu.tile([P, TCH], F32, name="pg", bufs=2)
                    for k in range(KT):
                        nc.tensor.matmul(
                            pg[:], wg_sb[:, k, ts(dd, P)], xt_sb[:, k, :],
                            start=(k == 0), stop=(k == KT - 1),
                        )
                    pu = pgu.tile([P, TCH], F32, name="pu", bufs=2)
                    for k in range(KT):
                        nc.tensor.matmul(
                            pu[:], wu_sb[:, k, ts(dd, P)], xt_sb[:, k, :],
                            start=(k == 0), stop=(k == KT - 1),
                        )
                    glu_t = gpool.tile([P, TCH], F16, name="glu_t")
                    if use_gelu_lut:
                        # glu = g*sigmoid(1.702 g), g = psum_gate + bg
                        nc.scalar.activation(
                            glu_t[:], pg[:], AF.Gelu_apprx_sigmoid,
                            bias=bg_sb[:, dd:dd + 1], scale=1.0,
                        )
                    else:
                        g_t = gpool.tile([P, TCH], F32, name="g_t")
                        nc.vector.tensor_scalar(
                            g_t[:], pg[:], bg_sb[:, dd:dd + 1], None, OP.add,
                        )
                        s_t = gpool.tile([P, TCH], F16, name="s_t")
                        nc.scalar.activation(s_t[:], g_t[:], AF.Sigmoid, scale=1.702)
                        nc.vector.tensor_mul(glu_t[:], g_t[:], s_t[:])
                    # act = (psum_up + (bu+1)) * glu
                    nc.vector.scalar_tensor_tensor(
                        act_t[:, dd, :], pu[:], bu1_sb[:, dd:dd + 1], glu_t[:],
                        OP.add, OP.mult,
                    )

            def emit_down(c):
                act_t = act_tiles[c]
                for tt in range(TTILES):
                    tcol = c * TTILES + tt
                    for hh in range(NHCH):
                        po = pdn.tile([P, HCH], F32, name="po", bufs=3)
                        for kd in range(KD):
                            nc.tensor.matmul(
                                po[:], act_t[:, kd, ts(tt, P)], wd_sb[:, kd, ts(hh, HCH)],
                                start=(kd == 0), stop=(kd == KD - 1),
                            )
                        # out = (psum + bd) * w_route[t]
                        qt = opool.tile([P, HCH], F32, name="qt")
                        nc.vector.tensor_add(qt[:], po[:], bdb_sb[:, ts(hh, HCH)])
                        ot = opool.tile([P, HCH], F32, name="ot")
                        nc.vector.tensor_scalar(
                            ot[:], qt[:], wr_sb[:, tcol:tcol + 1], None, OP.mult,
                        )
                        nc.sync.dma_start(
                            out_ap[ds(c * TCH + tt * P, P), ts(hh, HCH)], ot[:],
                        )

            xt3 = xt_d.ap().rearrange("(ko p) t -> p ko t", p=P)
            for c in range(NCH):
                xt_sb = xpool.tile([P, KT, TCH], F16, name="xt_sb")
                if c == 0:
                    # fine-grained startup: per-k slices so the first matmul
                    # only waits on its own k-slice of xt and wg
                    for k in range(KT):
                        nc.sync.dma_start(xt_sb[:, k, :], xt3[:, k, ts(c, TCH)])
                    for k in range(KT):
                        nc.sync.dma_start(wg_sb[:, k, :], wg3[:, k, :])
                    nc.sync.dma_start(wu_sb[:], wu3[:])
                else:
                    nc.sync.dma_start(xt_sb[:], xt3[:, :, ts(c, TCH)])
                emit_gateup(c, xt_sb)
                if c == 0:
                    nc.sync.dma_start(wd_sb[:], wd3[:])
                if c > 0:
                    emit_down(c - 1)
            emit_down(NCH - 1)

    nc.finalize()
    return nc


def make_in_maps(hidden_states, routing_weights, gate_up_proj, gate_up_proj_bias,
                 down_proj, down_proj_bias):
    T = hidden_states.shape[0]
    xt = np.ascontiguousarray(np.asarray(hidden_states, dtype=np.float32).T).astype(np.float16)
    gu = np.asarray(gate_up_proj, dtype=np.float32)
    gub = np.asarray(gate_up_proj_bias, dtype=np.float32)
    wd = np.asarray(down_proj, dtype=np.float32)
    bd = np.asarray(down_proj_bias, dtype=np.float32)
    wr = np.asarray(routing_weights, dtype=np.float32)

    in_maps = []
    for e in range(NUM_EXPERTS):
        in_maps.append({
            "xt": xt,
            "wg": np.ascontiguousarray(gu[e, :, 0::2]).astype(np.float16),
            "wu": np.ascontiguousarray(gu[e, :, 1::2]).astype(np.float16),
            "wd": np.ascontiguousarray(wd[e]).astype(np.float16),
            "bg": np.ascontiguousarray(gub[e, 0::2].reshape(D // P, P).T),
            "bu1": np.ascontiguousarray((gub[e, 1::2] + 1.0).reshape(D // P, P).T),
            "bdb": np.ascontiguousarray(np.broadcast_to(bd[e], (P, H))),
            "wr": np.ascontiguousarray(wr[:, e].reshape(T // P, P).T),
        })
    return in_maps


_NC_CACHE = {}


def _get_nc(T=4096):
    if T not in _NC_CACHE:
        _NC_CACHE[T] = build_nc(T)
    return _NC_CACHE[T]


def run(inputs, trace=False, trace_cores=None, **kwargs):
    """Build (cached), run on 8 cores, return (full_output, BassKernelResults)."""
    T = inputs["hidden_states"].shape[0]
    nc = _get_nc(T)
    in_maps = make_in_maps(**inputs)
    res = run_bass_kernel_spmd(
        nc, in_maps, core_ids=list(range(NUM_EXPERTS)),
        trace=trace, trace_cores=trace_cores, **kwargs,
    )
    out = np.zeros((T, H), np.float32)
    for c in range(NUM_EXPERTS):
        out += res.results[c]["out"]
    return out, res


def kernel(hidden_states, routing_weights, gate_up_proj, gate_up_proj_bias,
           down_proj, down_proj_bias):
    out, _ = run(dict(
        hidden_states=np.asarray(hidden_states),
        routing_weights=np.asarray(routing_weights),
        gate_up_proj=np.asarray(gate_up_proj),
        gate_up_proj_bias=np.asarray(gate_up_proj_bias),
        down_proj=np.asarray(down_proj),
        down_proj_bias=np.asarray(down_proj_bias),
    ))
    return out


# revision 11
# speedup vs baseline: 1.1026x; 1.0192x over previous
"""MoE experts kernel (GPT-OSS style, dense routing over 8 experts) for 8 trn2 NeuronCores.

Strategy: expert-parallel. Core e computes its expert's full MLP for all 4096
tokens, scaled by that expert's routing weight column; the host sums the 8
partial outputs (the unshard step for expert-parallel sharding).

Everything runs in a transposed [feature, token] layout on-chip so that no
transposes are needed anywhere:
  gate   = Wg^T @ X^T          (Wg [H,D] natural = lhsT, X^T [H,T] natural = rhs)
  up     = Wu^T @ X^T
  act    = (up + bu + 1) * gelu_sigmoid(gate + bg)     [D, T] fp16
  out    = (act^T @ Wd + ones^T bd) * w_route          (act tile = lhsT, Wd = rhs)
giving out in [T, H] layout directly.

Matmuls run in fp16 (4x the mantissa of bf16, same PE speed; all values here
are O(10) so fp16 range is ample). PSUM accumulation is fp32.
"""

import os
import numpy as np

import concourse.bass as bass
import concourse.mybir as mybir
import concourse.tile as tile
from concourse import bacc
from concourse.bass import ts, ds
from concourse.bass_utils import run_bass_kernel_spmd

AF = mybir.ActivationFunctionType
OP = mybir.AluOpType
F16 = mybir.dt.float16
F32 = mybir.dt.float32

P = 128
H = 1024          # hidden dim
D = 1024          # expert dim
NUM_EXPERTS = 8

# Gelu_apprx_sigmoid LUT computes x*sigmoid(1.702x) in one ACT pass.
# Fallback (False) builds it from Sigmoid + 2 DVE ops (simulator-friendly).
USE_GELU_LUT = True


def build_nc(T=4096, use_gelu_lut=USE_GELU_LUT):
    KT = H // P            # k-tiles for gate/up matmul (contraction over H)
    KD = D // P            # k-tiles for down matmul (contraction over D)
    DT = D // P            # d-tiles of the expert dim
    TCH = 512              # token chunk = psum free dim
    NCH = T // TCH         # token chunks
    TTILES = TCH // P      # 128-token tiles per chunk
    HCH = 512              # h chunk of the down matmul output
    NHCH = H // HCH

    nc = bacc.Bacc("TRN2", debug=False, enable_asserts=False, num_devices=NUM_EXPERTS)

    xt_d = nc.dram_tensor("xt", [H, T], F16, kind="ExternalInput")
    wg_d = nc.dram_tensor("wg", [H, D], F16, kind="ExternalInput")
    wu_d = nc.dram_tensor("wu", [H, D], F16, kind="ExternalInput")
    wd_d = nc.dram_tensor("wd", [D, H], F16, kind="ExternalInput")
    bg_d = nc.dram_tensor("bg", [P, DT], F32, kind="ExternalInput")
    bu1_d = nc.dram_tensor("bu1", [P, DT], F32, kind="ExternalInput")
    bdb_d = nc.dram_tensor("bdb", [P, H], F32, kind="ExternalInput")
    wr_d = nc.dram_tensor("wr", [P, T // P], F32, kind="ExternalInput")
    out_d = nc.dram_tensor("out", [T, H], F32, kind="ExternalOutput")
    out_ap = out_d.ap()

    with tile.TileContext(nc) as tc:
        with (
            tc.tile_pool(name="wpool", bufs=1) as wpool,
            tc.tile_pool(name="xpool", bufs=3) as xpool,
            tc.tile_pool(name="gpool", bufs=3) as gpool,
            tc.tile_pool(name="apool", bufs=2) as apool,
            tc.tile_pool(name="opool", bufs=4) as opool,
            tc.tile_pool(name="pgu", bufs=2, space="PSUM") as pgu,
            tc.tile_pool(name="pdn", bufs=3, space="PSUM") as pdn,
        ):
            # --- resident weights/biases ---
            # DMA *issue* costs ~0.6us each on the sync engine's queue, so
            # the emission order below is chosen to put the first matmul's
            # dependencies at the very front of the queue, and everything
            # else behind the point where it is first consumed.
            bg_sb = wpool.tile([P, DT], F32, name="bg_sb")
            bu1_sb = wpool.tile([P, DT], F32, name="bu1_sb")
            wr_sb = wpool.tile([P, T // P], F32, name="wr_sb")
            bdb_sb = wpool.tile([P, H], F32, name="bdb_sb")

            wg_sb = wpool.tile([P, KT, D], F16, name="wg_sb")
            wu_sb = wpool.tile([P, KT, D], F16, name="wu_sb")
            wd_sb = wpool.tile([P, KD, H], F16, name="wd_sb")
            wg3 = wg_d.ap().rearrange("(ko p) d -> p ko d", p=P)
            wu3 = wu_d.ap().rearrange("(ko p) d -> p ko d", p=P)
            wd3 = wd_d.ap().rearrange("(ko p) h -> p ko h", p=P)

            act_tiles = [None] * NCH
            GLU_BUFS = DT + 2

            def emit_gate_mms(dd, xt_sb):
                pg = pgu.tile([P, TCH], F32, name="pg", bufs=2)
                for k in range(KT):
                    nc.tensor.matmul(
                        pg[:], wg_sb[:, k, ts(dd, P)], xt_sb[:, k, :],
                        start=(k == 0), stop=(k == KT - 1),
                    )
                return pg

            def emit_glu(dd, pg):
                glu_t = gpool.tile([P, TCH], F16, name="glu_t", bufs=GLU_BUFS)
                if use_gelu_lut:
                    # glu = g*sigmoid(1.702 g), g = psum_gate + bg
                    nc.scalar.activation(
                        glu_t[:], pg[:], AF.Gelu_apprx_sigmoid,
                        bias=bg_sb[:, dd:dd + 1], scale=1.0,
                    )
                else:
                    g_t = gpool.tile([P, TCH], F32, name="g_t")
                    nc.vector.tensor_scalar(
                        g_t[:], pg[:], bg_sb[:, dd:dd + 1], None, OP.add,
                    )
                    s_t = gpool.tile([P, TCH], F16, name="s_t")
                    nc.scalar.activation(s_t[:], g_t[:], AF.Sigmoid, scale=1.702)
                    nc.vector.tensor_mul(glu_t[:], g_t[:], s_t[:])
                return glu_t

            def emit_up_act(dd, xt_sb, act_t, glu_t):
                pu = pgu.tile([P, TCH], F32, name="pu", bufs=2)
                for k in range(KT):
                    nc.tensor.matmul(
                        pu[:], wu_sb[:, k, ts(dd, P)], xt_sb[:, k, :],
                        start=(k == 0), stop=(k == KT - 1),
                    )
                # act = (psum_up + (bu+1)) * glu
                nc.vector.scalar_tensor_tensor(
                    act_t[:, dd, :], pu[:], bu1_sb[:, dd:dd + 1], glu_t[:],
                    OP.add, OP.mult,
                )

            def emit_gateup(c, xt_sb):
                act_t = apool.tile([P, DT, TCH], F16, name="act_t")
                act_tiles[c] = act_t
                for dd in range(DT):
                    pg = emit_gate_mms(dd, xt_sb)
                    glu_t = emit_glu(dd, pg)
                    emit_up_act(dd, xt_sb, act_t, glu_t)

            def emit_down(c):
                act_t = act_tiles[c]
                for tt in range(TTILES):
                    tcol = c * TTILES + tt
                    for hh in range(NHCH):
                        po = pdn.tile([P, HCH], F32, name="po", bufs=3)
                        for kd in range(KD):
                            nc.tensor.matmul(
                                po[:], act_t[:, kd, ts(tt, P)], wd_sb[:, kd, ts(hh, HCH)],
                                start=(kd == 0), stop=(kd == KD - 1),
                            )
                        # out = (psum + bd) * w_route[t]
                        qt = opool.tile([P, HCH], F32, name="qt")
                        nc.vector.tensor_add(qt[:], po[:], bdb_sb[:, ts(hh, HCH)])
                        ot = opool.tile([P, HCH], F32, name="ot")
                        nc.vector.tensor_scalar(
                            ot[:], qt[:], wr_sb[:, tcol:tcol + 1], None, OP.mult,
                        )
                        nc.sync.dma_start(
                            out_ap[ds(c * TCH + tt * P, P), ts(hh, HCH)], ot[:],
                        )

            xt3 = xt_d.ap().rearrange("(ko p) t -> p ko t", p=P)
            for c in range(NCH):
                xt_sb = xpool.tile([P, KT, TCH], F16, name="xt_sb")
                if c == 0:
                    # Startup choreography. Interleave per-k xt/wg slice DMAs
                    # so the k=0 gate matmul only sits behind two DMA issues;
                    # run the whole gate phase before the up phase so the wu
                    # DMA can issue+complete under the gate matmuls.
                    for k in range(KT):
                        nc.sync.dma_start(xt_sb[:, k, :], xt3[:, k, ts(c, TCH)])
                        nc.sync.dma_start(wg_sb[:, k, :], wg3[:, k, :])
                    act_t = apool.tile([P, DT, TCH], F16, name="act_t")
                    act_tiles[c] = act_t
                    pgs = [emit_gate_mms(0, xt_sb)]
                    # biases: needed first by the dd=0 glu drain (after 8 MMs)
                    nc.sync.dma_start(bg_sb[:], bg_d.ap())
                    nc.sync.dma_start(bu1_sb[:], bu1_d.ap())
                    glus = []
                    for dd in range(1, DT):
                        pgs.append(emit_gate_mms(dd, xt_sb))
                        glus.append(emit_glu(dd - 1, pgs[dd - 1]))
                    glus.append(emit_glu(DT - 1, pgs[DT - 1]))
                    # up weights: consumed right after the gate phase
                    nc.sync.dma_start(wu_sb[:], wu3[:])
                    # down-path constants: consumed by emit_down(0), a full
                    # chunk later
                    nc.sync.dma_start(wr_sb[:], wr_d.ap())
                    nc.sync.dma_start(bdb_sb[:], bdb_d.ap())
                    for dd in range(DT):
                        emit_up_act(dd, xt_sb, act_t, glus[dd])
                    nc.sync.dma_start(wd_sb[:], wd3[:])
                else:
                    nc.sync.dma_start(xt_sb[:], xt3[:, :, ts(c, TCH)])
                    emit_gateup(c, xt_sb)
                if c > 0:
                    emit_down(c - 1)
            emit_down(NCH - 1)

    nc.finalize()
    return nc


def make_in_maps(hidden_states, routing_weights, gate_up_proj, gate_up_proj_bias,
                 down_proj, down_proj_bias):
    T = hidden_states.shape[0]
    xt = np.ascontiguousarray(np.asarray(hidden_states, dtype=np.float32).T).astype(np.float16)
    gu = np.asarray(gate_up_proj, dtype=np.float32)
    gub = np.asarray(gate_up_proj_bias, dtype=np.float32)
    wd = np.asarray(down_proj, dtype=np.float32)
    bd = np.asarray(down_proj_bias, dtype=np.float32)
    wr = np.asarray(routing_weights, dtype=np.float32)

    in_maps = []
    for e in range(NUM_EXPERTS):
        in_maps.append({
            "xt": xt,
            "wg": np.ascontiguousarray(gu[e, :, 0::2]).astype(np.float16),
            "wu": np.ascontiguousarray(gu[e, :, 1::2]).astype(np.float16),
            "wd": np.ascontiguousarray(wd[e]).astype(np.float16),
            "bg": np.ascontiguousarray(gub[e, 0::2].reshape(D // P, P).T),
            "bu1": np.ascontiguousarray((gub[e, 1::2] + 1.0).reshape(D // P, P).T),
            "bdb": np.ascontiguousarray(np.broadcast_to(bd[e], (P, H))),
            "wr": np.ascontiguousarray(wr[:, e].reshape(T // P, P).T),
        })
    return in_maps


_NC_CACHE = {}


def _get_nc(T=4096):
    if T not in _NC_CACHE:
        _NC_CACHE[T] = build_nc(T)
    return _NC_CACHE[T]


def run(inputs, trace=False, trace_cores=None, **kwargs):
    """Build (cached), run on 8 cores, return (full_output, BassKernelResults)."""
    T = inputs["hidden_states"].shape[0]
    nc = _get_nc(T)
    in_maps = make_in_maps(**inputs)
    res = run_bass_kernel_spmd(
        nc, in_maps, core_ids=list(range(NUM_EXPERTS)),
        trace=trace, trace_cores=trace_cores, **kwargs,
    )
    out = np.zeros((T, H), np.float32)
    for c in range(NUM_EXPERTS):
        out += res.results[c]["out"]
    return out, res


def kernel(hidden_states, routing_weights, gate_up_proj, gate_up_proj_bias,
           down_proj, down_proj_bias):
    out, _ = run(dict(
        hidden_states=np.asarray(hidden_states),
        routing_weights=np.asarray(routing_weights),
        gate_up_proj=np.asarray(gate_up_proj),
        gate_up_proj_bias=np.asarray(gate_up_proj_bias),
        down_proj=np.asarray(down_proj),
        down_proj_bias=np.asarray(down_proj_bias),
    ))
    return out


# revision 12
# speedup vs baseline: 1.1029x; 1.0002x over previous
"""MoE experts kernel (GPT-OSS style, dense routing over 8 experts) for 8 trn2 NeuronCores.

Strategy: expert-parallel. Core e computes its expert's full MLP for all 4096
tokens, scaled by that expert's routing weight column; the host sums the 8
partial outputs (the unshard step for expert-parallel sharding).

Everything runs in a transposed [feature, token] layout on-chip so that no
transposes are needed anywhere:
  gate   = Wg^T @ X^T          (Wg [H,D] natural = lhsT, X^T [H,T] natural = rhs)
  up     = Wu^T @ X^T
  act    = (up + bu + 1) * gelu_sigmoid(gate + bg)     [D, T] fp16
  out    = (act^T @ Wd + ones^T bd) * w_route          (act tile = lhsT, Wd = rhs)
giving out in [T, H] layout directly.

Matmuls run in fp16 (4x the mantissa of bf16, same PE speed; all values here
are O(10) so fp16 range is ample). PSUM accumulation is fp32.
"""

import os
import numpy as np

import concourse.bass as bass
import concourse.mybir as mybir
import concourse.tile as tile
from concourse import bacc
from concourse.bass import ts, ds
from concourse.bass_utils import run_bass_kernel_spmd

AF = mybir.ActivationFunctionType
OP = mybir.AluOpType
F16 = mybir.dt.float16
F32 = mybir.dt.float32

P = 128
H = 1024          # hidden dim
D = 1024          # expert dim
NUM_EXPERTS = 8

# Gelu_apprx_sigmoid LUT computes x*sigmoid(1.702x) in one ACT pass.
# Fallback (False) builds it from Sigmoid + 2 DVE ops (simulator-friendly).
USE_GELU_LUT = True


def build_nc(T=4096, use_gelu_lut=USE_GELU_LUT):
    KT = H // P            # k-tiles for gate/up matmul (contraction over H)
    KD = D // P            # k-tiles for down matmul (contraction over D)
    DT = D // P            # d-tiles of the expert dim
    TCH = 512              # token chunk = psum free dim
    NCH = T // TCH         # token chunks
    TTILES = TCH // P      # 128-token tiles per chunk
    HCH = 512              # h chunk of the down matmul output
    NHCH = H // HCH

    nc = bacc.Bacc("TRN2", debug=False, enable_asserts=False, num_devices=NUM_EXPERTS)

    xt_d = nc.dram_tensor("xt", [H, T], F16, kind="ExternalInput")
    wg_d = nc.dram_tensor("wg", [H, D], F16, kind="ExternalInput")
    wu_d = nc.dram_tensor("wu", [H, D], F16, kind="ExternalInput")
    wd_d = nc.dram_tensor("wd", [D, H], F16, kind="ExternalInput")
    bg_d = nc.dram_tensor("bg", [P, DT], F32, kind="ExternalInput")
    bu1_d = nc.dram_tensor("bu1", [P, DT], F32, kind="ExternalInput")
    bdb_d = nc.dram_tensor("bdb", [P, H], F32, kind="ExternalInput")
    wr_d = nc.dram_tensor("wr", [P, T // P], F32, kind="ExternalInput")
    out_d = nc.dram_tensor("out", [T, H], F32, kind="ExternalOutput")
    out_ap = out_d.ap()

    with tile.TileContext(nc) as tc:
        with (
            tc.tile_pool(name="wpool", bufs=1) as wpool,
            tc.tile_pool(name="xpool", bufs=3) as xpool,
            tc.tile_pool(name="gpool", bufs=3) as gpool,
            tc.tile_pool(name="apool", bufs=2) as apool,
            tc.tile_pool(name="opool", bufs=4) as opool,
            tc.tile_pool(name="pgu", bufs=2, space="PSUM") as pgu,
            tc.tile_pool(name="pdn", bufs=3, space="PSUM") as pdn,
        ):
            # --- resident weights/biases ---
            # DMA *issue* costs ~0.6us each on the sync engine's queue, so
            # the emission order below is chosen to put the first matmul's
            # dependencies at the very front of the queue, and everything
            # else behind the point where it is first consumed.
            bg_sb = wpool.tile([P, DT], F32, name="bg_sb")
            bu1_sb = wpool.tile([P, DT], F32, name="bu1_sb")
            wr_sb = wpool.tile([P, T // P], F32, name="wr_sb")
            bdb_sb = wpool.tile([P, H], F32, name="bdb_sb")

            wg_sb = wpool.tile([P, KT, D], F16, name="wg_sb")
            wu_sb = wpool.tile([P, KT, D], F16, name="wu_sb")
            wd_sb = wpool.tile([P, KD, H], F16, name="wd_sb")
            wg3 = wg_d.ap().rearrange("(ko p) d -> p ko d", p=P)
            wu3 = wu_d.ap().rearrange("(ko p) d -> p ko d", p=P)
            wd3 = wd_d.ap().rearrange("(ko p) h -> p ko h", p=P)

            act_tiles = [None] * NCH
            GLU_BUFS = DT + 2

            def emit_gate_mms(dd, xt_sb):
                pg = pgu.tile([P, TCH], F32, name="pg", bufs=2)
                for k in range(KT):
                    nc.tensor.matmul(
                        pg[:], wg_sb[:, k, ts(dd, P)], xt_sb[:, k, :],
                        start=(k == 0), stop=(k == KT - 1),
                    )
                return pg

            def emit_glu(dd, pg):
                glu_t = gpool.tile([P, TCH], F16, name="glu_t", bufs=GLU_BUFS)
                if use_gelu_lut:
                    # glu = g*sigmoid(1.702 g), g = psum_gate + bg
                    nc.scalar.activation(
                        glu_t[:], pg[:], AF.Gelu_apprx_sigmoid,
                        bias=bg_sb[:, dd:dd + 1], scale=1.0,
                    )
                else:
                    g_t = gpool.tile([P, TCH], F32, name="g_t")
                    nc.vector.tensor_scalar(
                        g_t[:], pg[:], bg_sb[:, dd:dd + 1], None, OP.add,
                    )
                    s_t = gpool.tile([P, TCH], F16, name="s_t")
                    nc.scalar.activation(s_t[:], g_t[:], AF.Sigmoid, scale=1.702)
                    nc.vector.tensor_mul(glu_t[:], g_t[:], s_t[:])
                return glu_t

            def emit_up_act(dd, xt_sb, act_t, glu_t):
                pu = pgu.tile([P, TCH], F32, name="pu", bufs=2)
                for k in range(KT):
                    nc.tensor.matmul(
                        pu[:], wu_sb[:, k, ts(dd, P)], xt_sb[:, k, :],
                        start=(k == 0), stop=(k == KT - 1),
                    )
                # act = (psum_up + (bu+1)) * glu
                nc.vector.scalar_tensor_tensor(
                    act_t[:, dd, :], pu[:], bu1_sb[:, dd:dd + 1], glu_t[:],
                    OP.add, OP.mult,
                )

            def emit_gateup(c, xt_sb):
                act_t = apool.tile([P, DT, TCH], F16, name="act_t")
                act_tiles[c] = act_t
                for dd in range(DT):
                    pg = emit_gate_mms(dd, xt_sb)
                    glu_t = emit_glu(dd, pg)
                    emit_up_act(dd, xt_sb, act_t, glu_t)

            def emit_down(c):
                act_t = act_tiles[c]
                for tt in range(TTILES):
                    tcol = c * TTILES + tt
                    for hh in range(NHCH):
                        po = pdn.tile([P, HCH], F32, name="po", bufs=3)
                        for kd in range(KD):
                            nc.tensor.matmul(
                                po[:], act_t[:, kd, ts(tt, P)], wd_sb[:, kd, ts(hh, HCH)],
                                start=(kd == 0), stop=(kd == KD - 1),
                            )
                        # out = (psum + bd) * w_route[t]
                        qt = opool.tile([P, HCH], F32, name="qt")
                        nc.vector.tensor_add(qt[:], po[:], bdb_sb[:, ts(hh, HCH)])
                        ot = opool.tile([P, HCH], F32, name="ot")
                        nc.vector.tensor_scalar(
                            ot[:], qt[:], wr_sb[:, tcol:tcol + 1], None, OP.mult,
                        )
                        nc.sync.dma_start(
                            out_ap[ds(c * TCH + tt * P, P), ts(hh, HCH)], ot[:],
                        )

            xt3 = xt_d.ap().rearrange("(ko p) t -> p ko t", p=P)
            for c in range(NCH):
                xt_sb = xpool.tile([P, KT, TCH], F16, name="xt_sb")
                if c == 0:
                    # Startup choreography. Interleave per-k xt/wg slice DMAs
                    # so the k=0 gate matmul only sits behind two DMA issues;
                    # run the whole gate phase before the up phase so the wu
                    # DMA can issue+complete under the gate matmuls.
                    # two issue queues in parallel: xt via gpsimd, wg via sync
                    for k in range(KT):
                        nc.gpsimd.dma_start(xt_sb[:, k, :], xt3[:, k, ts(c, TCH)])
                        nc.sync.dma_start(wg_sb[:, k, :], wg3[:, k, :])
                    act_t = apool.tile([P, DT, TCH], F16, name="act_t")
                    act_tiles[c] = act_t
                    pgs = [emit_gate_mms(0, xt_sb)]
                    # biases: needed first by the dd=0 glu drain (after 8 MMs)
                    nc.gpsimd.dma_start(bg_sb[:], bg_d.ap())
                    nc.gpsimd.dma_start(bu1_sb[:], bu1_d.ap())
                    glus = []
                    for dd in range(1, DT):
                        pgs.append(emit_gate_mms(dd, xt_sb))
                        glus.append(emit_glu(dd - 1, pgs[dd - 1]))
                    glus.append(emit_glu(DT - 1, pgs[DT - 1]))
                    # up weights: consumed right after the gate phase
                    nc.sync.dma_start(wu_sb[:], wu3[:])
                    # down-path constants: consumed by emit_down(0), a full
                    # chunk later
                    nc.sync.dma_start(wr_sb[:], wr_d.ap())
                    nc.sync.dma_start(bdb_sb[:], bdb_d.ap())
                    for dd in range(DT):
                        emit_up_act(dd, xt_sb, act_t, glus[dd])
                    nc.sync.dma_start(wd_sb[:], wd3[:])
                else:
                    nc.sync.dma_start(xt_sb[:], xt3[:, :, ts(c, TCH)])
                    emit_gateup(c, xt_sb)
                if c > 0:
                    emit_down(c - 1)
            emit_down(NCH - 1)

    nc.finalize()
    return nc


def make_in_maps(hidden_states, routing_weights, gate_up_proj, gate_up_proj_bias,
                 down_proj, down_proj_bias):
    T = hidden_states.shape[0]
    xt = np.ascontiguousarray(np.asarray(hidden_states, dtype=np.float32).T).astype(np.float16)
    gu = np.asarray(gate_up_proj, dtype=np.float32)
    gub = np.asarray(gate_up_proj_bias, dtype=np.float32)
    wd = np.asarray(down_proj, dtype=np.float32)
    bd = np.asarray(down_proj_bias, dtype=np.float32)
    wr = np.asarray(routing_weights, dtype=np.float32)

    in_maps = []
    for e in range(NUM_EXPERTS):
        in_maps.append({
            "xt": xt,
            "wg": np.ascontiguousarray(gu[e, :, 0::2]).astype(np.float16),
            "wu": np.ascontiguousarray(gu[e, :, 1::2]).astype(np.float16),
            "wd": np.ascontiguousarray(wd[e]).astype(np.float16),
            "bg": np.ascontiguousarray(gub[e, 0::2].reshape(D // P, P).T),
            "bu1": np.ascontiguousarray((gub[e, 1::2] + 1.0).reshape(D // P, P).T),
            "bdb": np.ascontiguousarray(np.broadcast_to(bd[e], (P, H))),
            "wr": np.ascontiguousarray(wr[:, e].reshape(T // P, P).T),
        })
    return in_maps


_NC_CACHE = {}


def _get_nc(T=4096):
    if T not in _NC_CACHE:
        _NC_CACHE[T] = build_nc(T)
    return _NC_CACHE[T]


def run(inputs, trace=False, trace_cores=None, **kwargs):
    """Build (cached), run on 8 cores, return (full_output, BassKernelResults)."""
    T = inputs["hidden_states"].shape[0]
    nc = _get_nc(T)
    in_maps = make_in_maps(**inputs)
    res = run_bass_kernel_spmd(
        nc, in_maps, core_ids=list(range(NUM_EXPERTS)),
        trace=trace, trace_cores=trace_cores, **kwargs,
    )
    out = np.zeros((T, H), np.float32)
    for c in range(NUM_EXPERTS):
        out += res.results[c]["out"]
    return out, res


def kernel(hidden_states, routing_weights, gate_up_proj, gate_up_proj_bias,
           down_proj, down_proj_bias):
    out, _ = run(dict(
        hidden_states=np.asarray(hidden_states),
        routing_weights=np.asarray(routing_weights),
        gate_up_proj=np.asarray(gate_up_proj),
        gate_up_proj_bias=np.asarray(gate_up_proj_bias),
        down_proj=np.asarray(down_proj),
        down_proj_bias=np.asarray(down_proj_bias),
    ))
    return out


# revision 15
# speedup vs baseline: 1.1157x; 1.0116x over previous
"""MoE experts kernel (GPT-OSS style, dense routing over 8 experts) for 8 trn2 NeuronCores.

Strategy: expert-parallel. Core e computes its expert's full MLP for all 4096
tokens, scaled by that expert's routing weight column; the host sums the 8
partial outputs (the unshard step for expert-parallel sharding).

Everything runs in a transposed [feature, token] layout on-chip so that no
transposes are needed anywhere:
  gate   = Wg^T @ X^T          (Wg [H,D] natural = lhsT, X^T [H,T] natural = rhs)
  up     = Wu^T @ X^T
  act    = (up + bu + 1) * gelu_sigmoid(gate + bg)     [D, T] fp16
  out    = (act^T @ Wd + ones^T bd) * w_route          (act tile = lhsT, Wd = rhs)
giving out in [T, H] layout directly.

Matmuls run in fp16 (4x the mantissa of bf16, same PE speed; all values here
are O(10) so fp16 range is ample). PSUM accumulation is fp32.
"""

import os
import numpy as np

import concourse.bass as bass
import concourse.mybir as mybir
import concourse.tile as tile
from concourse import bacc
from concourse.bass import ts, ds
from concourse.bass_utils import run_bass_kernel_spmd

AF = mybir.ActivationFunctionType
OP = mybir.AluOpType
F16 = mybir.dt.float16
F32 = mybir.dt.float32

P = 128
H = 1024          # hidden dim
D = 1024          # expert dim
NUM_EXPERTS = 8

# Gelu_apprx_sigmoid LUT computes x*sigmoid(1.702x) in one ACT pass.
# Fallback (False) builds it from Sigmoid + 2 DVE ops (simulator-friendly).
USE_GELU_LUT = True


def build_nc(T=4096, use_gelu_lut=USE_GELU_LUT):
    KT = H // P            # k-tiles for gate/up matmul (contraction over H)
    KD = D // P            # k-tiles for down matmul (contraction over D)
    DT = D // P            # d-tiles of the expert dim
    TCH = 512              # token chunk = psum free dim
    NCH = T // TCH         # token chunks
    TTILES = TCH // P      # 128-token tiles per chunk
    HCH = 512              # h chunk of the down matmul output
    NHCH = H // HCH

    nc = bacc.Bacc("TRN2", debug=False, enable_asserts=False, num_devices=NUM_EXPERTS)

    xt_d = nc.dram_tensor("xt", [H, T], F16, kind="ExternalInput")
    wg_d = nc.dram_tensor("wg", [H, D], F16, kind="ExternalInput")
    wu_d = nc.dram_tensor("wu", [H, D], F16, kind="ExternalInput")
    wd_d = nc.dram_tensor("wd", [D, H], F16, kind="ExternalInput")
    bg_d = nc.dram_tensor("bg", [P, DT], F32, kind="ExternalInput")
    bu1_d = nc.dram_tensor("bu1", [P, DT], F32, kind="ExternalInput")
    bdb_d = nc.dram_tensor("bdb", [P, H], F32, kind="ExternalInput")
    wr_d = nc.dram_tensor("wr", [P, T // P], F32, kind="ExternalInput")
    out_d = nc.dram_tensor("out", [T, H], F32, kind="ExternalOutput")
    out_ap = out_d.ap()

    with tile.TileContext(nc) as tc:
        with (
            tc.tile_pool(name="wpool", bufs=1) as wpool,
            tc.tile_pool(name="xpool", bufs=3) as xpool,
            tc.tile_pool(name="gpool", bufs=3) as gpool,
            tc.tile_pool(name="apool", bufs=2) as apool,
            tc.tile_pool(name="opool", bufs=4) as opool,
            tc.tile_pool(name="pgu", bufs=2, space="PSUM") as pgu,
            tc.tile_pool(name="pdn", bufs=3, space="PSUM") as pdn,
        ):
            # --- resident weights/biases ---
            # DMA *issue* costs ~0.6us each on the sync engine's queue, so
            # the emission order below is chosen to put the first matmul's
            # dependencies at the very front of the queue, and everything
            # else behind the point where it is first consumed.
            bg_sb = wpool.tile([P, DT], F32, name="bg_sb")
            bu1_sb = wpool.tile([P, DT], F32, name="bu1_sb")
            wr_sb = wpool.tile([P, T // P], F32, name="wr_sb")
            bdb_sb = wpool.tile([P, H], F32, name="bdb_sb")

            wg_sb = wpool.tile([P, KT, D], F16, name="wg_sb")
            wu_sb = wpool.tile([P, KT, D], F16, name="wu_sb")
            wd_sb = wpool.tile([P, KD, H], F16, name="wd_sb")
            wg3 = wg_d.ap().rearrange("(ko p) d -> p ko d", p=P)
            wu3 = wu_d.ap().rearrange("(ko p) d -> p ko d", p=P)
            wd3 = wd_d.ap().rearrange("(ko p) h -> p ko h", p=P)

            act_tiles = [None] * NCH
            GLU_BUFS = DT + 2

            def emit_gate_mms(dd, xt_sb):
                pg = pgu.tile([P, TCH], F32, name="pg", bufs=4)
                for k in range(KT):
                    nc.tensor.matmul(
                        pg[:], wg_sb[:, k, ts(dd, P)], xt_sb[:, k, :],
                        start=(k == 0), stop=(k == KT - 1),
                    )
                return pg

            def emit_glu(dd, pg):
                glu_t = gpool.tile([P, TCH], F16, name="glu_t", bufs=GLU_BUFS)
                if use_gelu_lut:
                    # glu = g*sigmoid(1.702 g), g = psum_gate + bg
                    nc.scalar.activation(
                        glu_t[:], pg[:], AF.Gelu_apprx_sigmoid,
                        bias=bg_sb[:, dd:dd + 1], scale=1.0,
                    )
                else:
                    g_t = gpool.tile([P, TCH], F32, name="g_t")
                    nc.vector.tensor_scalar(
                        g_t[:], pg[:], bg_sb[:, dd:dd + 1], None, OP.add,
                    )
                    s_t = gpool.tile([P, TCH], F16, name="s_t")
                    nc.scalar.activation(s_t[:], g_t[:], AF.Sigmoid, scale=1.702)
                    nc.vector.tensor_mul(glu_t[:], g_t[:], s_t[:])
                return glu_t

            def emit_up_act(dd, xt_sb, act_t, glu_t):
                pu = pgu.tile([P, TCH], F32, name="pu", bufs=2)
                for k in range(KT):
                    nc.tensor.matmul(
                        pu[:], wu_sb[:, k, ts(dd, P)], xt_sb[:, k, :],
                        start=(k == 0), stop=(k == KT - 1),
                    )
                # act = (psum_up + (bu+1)) * glu
                nc.vector.scalar_tensor_tensor(
                    act_t[:, dd, :], pu[:], bu1_sb[:, dd:dd + 1], glu_t[:],
                    OP.add, OP.mult,
                )

            def emit_gateup(c, xt_sb):
                act_t = apool.tile([P, DT, TCH], F16, name="act_t")
                act_tiles[c] = act_t
                for dd in range(DT):
                    pg = emit_gate_mms(dd, xt_sb)
                    glu_t = emit_glu(dd, pg)
                    emit_up_act(dd, xt_sb, act_t, glu_t)

            def emit_down(c):
                act_t = act_tiles[c]
                for tt in range(TTILES):
                    tcol = c * TTILES + tt
                    for hh in range(NHCH):
                        po = pdn.tile([P, HCH], F32, name="po", bufs=2)
                        for kd in range(KD):
                            nc.tensor.matmul(
                                po[:], act_t[:, kd, ts(tt, P)], wd_sb[:, kd, ts(hh, HCH)],
                                start=(kd == 0), stop=(kd == KD - 1),
                            )
                        # out = (psum + bd) * w_route[t]
                        qt = opool.tile([P, HCH], F32, name="qt")
                        nc.vector.tensor_add(qt[:], po[:], bdb_sb[:, ts(hh, HCH)])
                        ot = opool.tile([P, HCH], F32, name="ot")
                        nc.vector.tensor_scalar(
                            ot[:], qt[:], wr_sb[:, tcol:tcol + 1], None, OP.mult,
                        )
                        nc.sync.dma_start(
                            out_ap[ds(c * TCH + tt * P, P), ts(hh, HCH)], ot[:],
                        )

            xt3 = xt_d.ap().rearrange("(ko p) t -> p ko t", p=P)
            for c in range(NCH):
                xt_sb = xpool.tile([P, KT, TCH], F16, name="xt_sb")
                if c == 0:
                    # Startup choreography. The matmul stream becomes dense as
                    # soon as the first k-slices land: the gate phase runs
                    # k-outer over dd-groups of 4 (4 psum banks), so each
                    # arriving (wg_k, xt) slice immediately feeds 4 matmuls.
                    nc.sync.dma_start(wg_sb[:, 0, :], wg3[:, 0, :])
                    nc.sync.dma_start(xt_sb[:, 0:4, :], xt3[:, 0:4, ts(c, TCH)])
                    nc.sync.dma_start(xt_sb[:, 4:8, :], xt3[:, 4:8, ts(c, TCH)])
                    for k in range(1, KT):
                        nc.sync.dma_start(wg_sb[:, k, :], wg3[:, k, :])
                    # biases via the gpsimd queue (needed by the glu drains)
                    nc.gpsimd.dma_start(bg_sb[:], bg_d.ap())
                    nc.gpsimd.dma_start(bu1_sb[:], bu1_d.ap())
                    act_t = apool.tile([P, DT, TCH], F16, name="act_t")
                    act_tiles[c] = act_t
                    glus = [None] * DT
                    for g in range(2):
                        dds = list(range(4 * g, 4 * g + 4))
                        pgs4 = [pgu.tile([P, TCH], F32, name="pg", bufs=4)
                                for _ in dds]
                        for k in range(KT):
                            for i, dd in enumerate(dds):
                                nc.tensor.matmul(
                                    pgs4[i][:], wg_sb[:, k, ts(dd, P)], xt_sb[:, k, :],
                                    start=(k == 0), stop=(k == KT - 1),
                                )
                        if g == 0:
                            # up weights: consumed right after the gate phase
                            nc.sync.dma_start(wu_sb[:], wu3[:])
                        for i, dd in enumerate(dds):
                            glus[dd] = emit_glu(dd, pgs4[i])
                    # down-path constants: consumed by emit_down(0)
                    nc.sync.dma_start(wr_sb[:], wr_d.ap())
                    nc.sync.dma_start(bdb_sb[:], bdb_d.ap())
                    for dd in range(DT):
                        emit_up_act(dd, xt_sb, act_t, glus[dd])
                    nc.sync.dma_start(wd_sb[:], wd3[:])
                else:
                    nc.sync.dma_start(xt_sb[:], xt3[:, :, ts(c, TCH)])
                    emit_gateup(c, xt_sb)
                if c > 0:
                    emit_down(c - 1)
            emit_down(NCH - 1)

    nc.finalize()
    return nc


def make_in_maps(hidden_states, routing_weights, gate_up_proj, gate_up_proj_bias,
                 down_proj, down_proj_bias):
    T = hidden_states.shape[0]
    xt = np.ascontiguousarray(np.asarray(hidden_states, dtype=np.float32).T).astype(np.float16)
    gu = np.asarray(gate_up_proj, dtype=np.float32)
    gub = np.asarray(gate_up_proj_bias, dtype=np.float32)
    wd = np.asarray(down_proj, dtype=np.float32)
    bd = np.asarray(down_proj_bias, dtype=np.float32)
    wr = np.asarray(routing_weights, dtype=np.float32)

    in_maps = []
    for e in range(NUM_EXPERTS):
        in_maps.append({
            "xt": xt,
            "wg": np.ascontiguousarray(gu[e, :, 0::2]).astype(np.float16),
            "wu": np.ascontiguousarray(gu[e, :, 1::2]).astype(np.float16),
            "wd": np.ascontiguousarray(wd[e]).astype(np.float16),
            "bg": np.ascontiguousarray(gub[e, 0::2].reshape(D // P, P).T),
            "bu1": np.ascontiguousarray((gub[e, 1::2] + 1.0).reshape(D // P, P).T),
            "bdb": np.ascontiguousarray(np.broadcast_to(bd[e], (P, H))),
            "wr": np.ascontiguousarray(wr[:, e].reshape(T // P, P).T),
        })
    return in_maps


_NC_CACHE = {}


def _get_nc(T=4096):
    if T not in _NC_CACHE:
        _NC_CACHE[T] = build_nc(T)
    return _NC_CACHE[T]


def run(inputs, trace=False, trace_cores=None, **kwargs):
    """Build (cached), run on 8 cores, return (full_output, BassKernelResults)."""
    T = inputs["hidden_states"].shape[0]
    nc = _get_nc(T)
    in_maps = make_in_maps(**inputs)
    res = run_bass_kernel_spmd(
        nc, in_maps, core_ids=list(range(NUM_EXPERTS)),
        trace=trace, trace_cores=trace_cores, **kwargs,
    )
    out = np.zeros((T, H), np.float32)
    for c in range(NUM_EXPERTS):
        out += res.results[c]["out"]
    return out, res


def kernel(hidden_states, routing_weights, gate_up_proj, gate_up_proj_bias,
           down_proj, down_proj_bias):
    out, _ = run(dict(
        hidden_states=np.asarray(hidden_states),
        routing_weights=np.asarray(routing_weights),
        gate_up_proj=np.asarray(gate_up_proj),
        gate_up_proj_bias=np.asarray(gate_up_proj_bias),
        down_proj=np.asarray(down_proj),
        down_proj_bias=np.asarray(down_proj_bias),
    ))
    return out


# revision 16
# speedup vs baseline: 1.1211x; 1.0048x over previous
"""MoE experts kernel (GPT-OSS style, dense routing over 8 experts) for 8 trn2 NeuronCores.

Strategy: expert-parallel. Core e computes its expert's full MLP for all 4096
tokens, scaled by that expert's routing weight column; the host sums the 8
partial outputs (the unshard step for expert-parallel sharding).

Everything runs in a transposed [feature, token] layout on-chip so that no
transposes are needed anywhere:
  gate   = Wg^T @ X^T          (Wg [H,D] natural = lhsT, X^T [H,T] natural = rhs)
  up     = Wu^T @ X^T
  act    = (up + bu + 1) * gelu_sigmoid(gate + bg)     [D, T] fp16
  out    = (act^T @ Wd + ones^T bd) * w_route          (act tile = lhsT, Wd = rhs)
giving out in [T, H] layout directly.

Matmuls run in fp16 (4x the mantissa of bf16, same PE speed; all values here
are O(10) so fp16 range is ample). PSUM accumulation is fp32.
"""

import os
import numpy as np

import concourse.bass as bass
import concourse.mybir as mybir
import concourse.tile as tile
from concourse import bacc
from concourse.bass import ts, ds
from concourse.bass_utils import run_bass_kernel_spmd

AF = mybir.ActivationFunctionType
OP = mybir.AluOpType
F16 = mybir.dt.float16
F32 = mybir.dt.float32

P = 128
H = 1024          # hidden dim
D = 1024          # expert dim
NUM_EXPERTS = 8

# Gelu_apprx_sigmoid LUT computes x*sigmoid(1.702x) in one ACT pass.
# Fallback (False) builds it from Sigmoid + 2 DVE ops (simulator-friendly).
USE_GELU_LUT = True


def build_nc(T=4096, use_gelu_lut=USE_GELU_LUT):
    KT = H // P            # k-tiles for gate/up matmul (contraction over H)
    KD = D // P            # k-tiles for down matmul (contraction over D)
    DT = D // P            # d-tiles of the expert dim
    TCH = 512              # token chunk = psum free dim
    NCH = T // TCH         # token chunks
    TTILES = TCH // P      # 128-token tiles per chunk
    HCH = 512              # h chunk of the down matmul output
    NHCH = H // HCH

    nc = bacc.Bacc("TRN2", debug=False, enable_asserts=False, num_devices=NUM_EXPERTS)

    xt_d = nc.dram_tensor("xt", [H, T], F16, kind="ExternalInput")
    wg_d = nc.dram_tensor("wg", [H, D], F16, kind="ExternalInput")
    wu_d = nc.dram_tensor("wu", [H, D], F16, kind="ExternalInput")
    wd_d = nc.dram_tensor("wd", [D, H], F16, kind="ExternalInput")
    bg_d = nc.dram_tensor("bg", [P, DT], F32, kind="ExternalInput")
    bu1_d = nc.dram_tensor("bu1", [P, DT], F32, kind="ExternalInput")
    bdb_d = nc.dram_tensor("bdb", [P, H], F32, kind="ExternalInput")
    wr_d = nc.dram_tensor("wr", [P, T // P], F32, kind="ExternalInput")
    out_d = nc.dram_tensor("out", [T, H], F32, kind="ExternalOutput")
    out_ap = out_d.ap()

    with tile.TileContext(nc) as tc:
        with (
            tc.tile_pool(name="wpool", bufs=1) as wpool,
            tc.tile_pool(name="xpool", bufs=3) as xpool,
            tc.tile_pool(name="gpool", bufs=3) as gpool,
            tc.tile_pool(name="apool", bufs=2) as apool,
            tc.tile_pool(name="opool", bufs=4) as opool,
            tc.tile_pool(name="pgu", bufs=2, space="PSUM") as pgu,
            tc.tile_pool(name="pdn", bufs=3, space="PSUM") as pdn,
        ):
            # --- resident weights/biases ---
            # DMA *issue* costs ~0.6us each on the sync engine's queue, so
            # the emission order below is chosen to put the first matmul's
            # dependencies at the very front of the queue, and everything
            # else behind the point where it is first consumed.
            bg_sb = wpool.tile([P, DT], F32, name="bg_sb")
            bu1_sb = wpool.tile([P, DT], F32, name="bu1_sb")
            wr_sb = wpool.tile([P, T // P], F32, name="wr_sb")
            bdb_sb = wpool.tile([P, H], F32, name="bdb_sb")

            wg_sb = wpool.tile([P, KT, D], F16, name="wg_sb")
            wu_sb = wpool.tile([P, KT, D], F16, name="wu_sb")
            wd_sb = wpool.tile([P, KD, H], F16, name="wd_sb")
            wg3 = wg_d.ap().rearrange("(ko p) d -> p ko d", p=P)
            wu3 = wu_d.ap().rearrange("(ko p) d -> p ko d", p=P)
            wd3 = wd_d.ap().rearrange("(ko p) h -> p ko h", p=P)

            act_tiles = [None] * NCH
            GLU_BUFS = DT + 2

            def emit_gate_mms(dd, xt_sb):
                pg = pgu.tile([P, TCH], F32, name="pg", bufs=4)
                for k in range(KT):
                    nc.tensor.matmul(
                        pg[:], wg_sb[:, k, ts(dd, P)], xt_sb[:, k, :],
                        start=(k == 0), stop=(k == KT - 1),
                    )
                return pg

            def emit_glu(dd, pg):
                glu_t = gpool.tile([P, TCH], F16, name="glu_t", bufs=GLU_BUFS)
                if use_gelu_lut:
                    # glu = g*sigmoid(1.702 g), g = psum_gate + bg
                    nc.scalar.activation(
                        glu_t[:], pg[:], AF.Gelu_apprx_sigmoid,
                        bias=bg_sb[:, dd:dd + 1], scale=1.0,
                    )
                else:
                    g_t = gpool.tile([P, TCH], F32, name="g_t")
                    nc.vector.tensor_scalar(
                        g_t[:], pg[:], bg_sb[:, dd:dd + 1], None, OP.add,
                    )
                    s_t = gpool.tile([P, TCH], F16, name="s_t")
                    nc.scalar.activation(s_t[:], g_t[:], AF.Sigmoid, scale=1.702)
                    nc.vector.tensor_mul(glu_t[:], g_t[:], s_t[:])
                return glu_t

            def emit_up_act(dd, xt_sb, act_t, glu_t):
                pu = pgu.tile([P, TCH], F32, name="pu", bufs=2)
                for k in range(KT):
                    nc.tensor.matmul(
                        pu[:], wu_sb[:, k, ts(dd, P)], xt_sb[:, k, :],
                        start=(k == 0), stop=(k == KT - 1),
                    )
                # act = (psum_up + (bu+1)) * glu
                nc.vector.scalar_tensor_tensor(
                    act_t[:, dd, :], pu[:], bu1_sb[:, dd:dd + 1], glu_t[:],
                    OP.add, OP.mult,
                )

            def emit_gateup(c, xt_sb):
                act_t = apool.tile([P, DT, TCH], F16, name="act_t")
                act_tiles[c] = act_t
                for dd in range(DT):
                    pg = emit_gate_mms(dd, xt_sb)
                    glu_t = emit_glu(dd, pg)
                    emit_up_act(dd, xt_sb, act_t, glu_t)

            def emit_down(c):
                act_t = act_tiles[c]
                for tt in range(TTILES):
                    tcol = c * TTILES + tt
                    for hh in range(NHCH):
                        po = pdn.tile([P, HCH], F32, name="po", bufs=2)
                        for kd in range(KD):
                            nc.tensor.matmul(
                                po[:], act_t[:, kd, ts(tt, P)], wd_sb[:, kd, ts(hh, HCH)],
                                start=(kd == 0), stop=(kd == KD - 1),
                            )
                        # out = (psum + bd) * w_route[t]
                        qt = opool.tile([P, HCH], F32, name="qt")
                        nc.vector.tensor_add(qt[:], po[:], bdb_sb[:, ts(hh, HCH)])
                        ot = opool.tile([P, HCH], F32, name="ot")
                        nc.vector.tensor_scalar(
                            ot[:], qt[:], wr_sb[:, tcol:tcol + 1], None, OP.mult,
                        )
                        nc.sync.dma_start(
                            out_ap[ds(c * TCH + tt * P, P), ts(hh, HCH)], ot[:],
                        )

            xt3 = xt_d.ap().rearrange("(ko p) t -> p ko t", p=P)
            for c in range(NCH):
                xt_sb = xpool.tile([P, KT, TCH], F16, name="xt_sb")
                if c == 0:
                    # Startup choreography. The matmul stream becomes dense as
                    # soon as the first k-slices land: the gate phase runs
                    # k-outer over dd-groups of 4 (4 psum banks), so each
                    # arriving (wg_k, xt) slice immediately feeds 4 matmuls.
                    nc.sync.dma_start(wg_sb[:, 0, :], wg3[:, 0, :])
                    nc.sync.dma_start(xt_sb[:, 0, :], xt3[:, 0, ts(c, TCH)])
                    nc.sync.dma_start(wg_sb[:, 1, :], wg3[:, 1, :])
                    nc.sync.dma_start(xt_sb[:, 1:4, :], xt3[:, 1:4, ts(c, TCH)])
                    nc.sync.dma_start(wg_sb[:, 2, :], wg3[:, 2, :])
                    nc.sync.dma_start(wg_sb[:, 3, :], wg3[:, 3, :])
                    nc.sync.dma_start(xt_sb[:, 4:8, :], xt3[:, 4:8, ts(c, TCH)])
                    for k in range(4, KT):
                        nc.sync.dma_start(wg_sb[:, k, :], wg3[:, k, :])
                    # biases via the gpsimd queue (needed by the glu drains)
                    nc.gpsimd.dma_start(bg_sb[:], bg_d.ap())
                    nc.gpsimd.dma_start(bu1_sb[:], bu1_d.ap())
                    act_t = apool.tile([P, DT, TCH], F16, name="act_t")
                    act_tiles[c] = act_t
                    glus = [None] * DT
                    for g in range(2):
                        dds = list(range(4 * g, 4 * g + 4))
                        pgs4 = [pgu.tile([P, TCH], F32, name="pg", bufs=4)
                                for _ in dds]
                        for k in range(KT):
                            for i, dd in enumerate(dds):
                                nc.tensor.matmul(
                                    pgs4[i][:], wg_sb[:, k, ts(dd, P)], xt_sb[:, k, :],
                                    start=(k == 0), stop=(k == KT - 1),
                                )
                        if g == 0:
                            # up weights: consumed right after the gate phase
                            nc.sync.dma_start(wu_sb[:], wu3[:])
                        for i, dd in enumerate(dds):
                            glus[dd] = emit_glu(dd, pgs4[i])
                    # down-path constants: consumed by emit_down(0)
                    nc.sync.dma_start(wr_sb[:], wr_d.ap())
                    nc.sync.dma_start(bdb_sb[:], bdb_d.ap())
                    for dd in range(DT):
                        emit_up_act(dd, xt_sb, act_t, glus[dd])
                    nc.sync.dma_start(wd_sb[:], wd3[:])
                else:
                    nc.sync.dma_start(xt_sb[:], xt3[:, :, ts(c, TCH)])
                    emit_gateup(c, xt_sb)
                if c > 0:
                    emit_down(c - 1)
            emit_down(NCH - 1)

    nc.finalize()
    return nc


def make_in_maps(hidden_states, routing_weights, gate_up_proj, gate_up_proj_bias,
                 down_proj, down_proj_bias):
    T = hidden_states.shape[0]
    xt = np.ascontiguousarray(np.asarray(hidden_states, dtype=np.float32).T).astype(np.float16)
    gu = np.asarray(gate_up_proj, dtype=np.float32)
    gub = np.asarray(gate_up_proj_bias, dtype=np.float32)
    wd = np.asarray(down_proj, dtype=np.float32)
    bd = np.asarray(down_proj_bias, dtype=np.float32)
    wr = np.asarray(routing_weights, dtype=np.float32)

    in_maps = []
    for e in range(NUM_EXPERTS):
        in_maps.append({
            "xt": xt,
            "wg": np.ascontiguousarray(gu[e, :, 0::2]).astype(np.float16),
            "wu": np.ascontiguousarray(gu[e, :, 1::2]).astype(np.float16),
            "wd": np.ascontiguousarray(wd[e]).astype(np.float16),
            "bg": np.ascontiguousarray(gub[e, 0::2].reshape(D // P, P).T),
            "bu1": np.ascontiguousarray((gub[e, 1::2] + 1.0).reshape(D // P, P).T),
            "bdb": np.ascontiguousarray(np.broadcast_to(bd[e], (P, H))),
            "wr": np.ascontiguousarray(wr[:, e].reshape(T // P, P).T),
        })
    return in_maps


_NC_CACHE = {}


def _get_nc(T=4096):
    if T not in _NC_CACHE:
        _NC_CACHE[T] = build_nc(T)
    return _NC_CACHE[T]


def run(inputs, trace=False, trace_cores=None, **kwargs):
    """Build (cached), run on 8 cores, return (full_output, BassKernelResults)."""
    T = inputs["hidden_states"].shape[0]
    nc = _get_nc(T)
    in_maps = make_in_maps(**inputs)
    res = run_bass_kernel_spmd(
        nc, in_maps, core_ids=list(range(NUM_EXPERTS)),
        trace=trace, trace_cores=trace_cores, **kwargs,
    )
    out = np.zeros((T, H), np.float32)
    for c in range(NUM_EXPERTS):
        out += res.results[c]["out"]
    return out, res


def kernel(hidden_states, routing_weights, gate_up_proj, gate_up_proj_bias,
           down_proj, down_proj_bias):
    out, _ = run(dict(
        hidden_states=np.asarray(hidden_states),
        routing_weights=np.asarray(routing_weights),
        gate_up_proj=np.asarray(gate_up_proj),
        gate_up_proj_bias=np.asarray(gate_up_proj_bias),
        down_proj=np.asarray(down_proj),
        down_proj_bias=np.asarray(down_proj_bias),
    ))
    return out
